# revision 14
# baseline (speedup 1.0000x reference)
"""BiLSTM language model kernel for Trainium2 (8 NeuronCores).

Sharding: data-parallel over batch (B=32 -> 4 per core). Each core runs the
full bidirectional LSTM scan for its batch slice and computes the full-vocab
output projection + log-softmax for its tokens locally (no collectives).

Structure per core:
  - embedding rows gathered by indirect DMA, transposed via PE into a [32, T]
    "embT"; a single [48, 520] "comb" tile holds x (rows 0-31) and the h state
    (rows 32-47) for both scan directions; its h rows after the scan ARE the
    lefts/rights features.
  - fused bidirectional scan: one matmul + 3 ACT + 4 DVE ops per step handle
    both directions at once (each op acts on [*, 8] = fwd|rev batch columns).
  - output projection in two passes over vocab chunks, bf16 matmuls (PE fp32
    streams at 1/4 rate), K=33 with a ones-row paired against b_ho so the
    bias rides in the matmul. Pass A computes sum(exp) via the ACT accum_out
    port; pass B re-runs the matmuls and applies the fp32 -log(sum(exp)) as
    per-partition bias during the PSUM->SBUF copy, then DMAs out.
  - token tiles are middle-out: tile M = s 16..47 is finished by both scan
    directions ~3/4 through the scan, so its projection overlaps the scan
    tail; tile O = s {0..15, 48..63} follows. Phases pipeline as
    A(M) -> B(M) || A(O) -> B(O).

DVE/ACT constraints honored throughout: access patterns must start at
32-aligned partitions, and 2-tensor DVE ops need both SBUF inputs at the
same base partition (gates land as {f@0,i@32,o@64} with tanh intermediates
parked at matching bases).
"""

import numpy as np
from contextlib import ExitStack

import concourse.bass as bass
import concourse.mybir as mybir
import concourse.tile as tile
from concourse import bacc
from concourse.masks import make_identity

F32 = mybir.dt.float32
BF16 = mybir.dt.bfloat16
AF = mybir.ActivationFunctionType
ALU = mybir.AluOpType

S = 64          # sequence length
B = 32          # full batch
V = 50257       # vocab
HID = 16
EMB = 32
NCORES = 8
BL = B // NCORES          # batch per core = 4
T = S * BL                # tokens per core = 256
KC = EMB + HID            # 48
GP = 128                  # padded gate rows (f@0, i@32, o@64, c@96)
REV = (S + 1) * BL        # column offset of reverse region in comb = 260
GROUP = 1536              # vocab elements per psum group (3 banks)
HALF = 16 * BL            # 64 columns = 16 s-steps


def _two_block(ap2d, col_a, col_b, width):
    """AP selecting two `width`-column blocks [P, 2, width] of a 2D sbuf AP."""
    base = ap2d
    return bass.AP(
        base.tensor,
        base.offset + col_a,
        [base.ap[0], [col_b - col_a, 2], [1, width]],
    )


def build_nc(v=V):
    """Build the per-core Bass module. `v` is overridable for simulator tests."""
    nc = bacc.Bacc("TRN2", target_bir_lowering=False, debug=False)

    # ---------------- DRAM I/O ----------------
    d_emb = nc.dram_tensor("emb_table", [v, EMB], F32, kind="ExternalInput")
    d_idx = nc.dram_tensor("idx", [T, 1], mybir.dt.int32, kind="ExternalInput")
    d_wcomb = nc.dram_tensor("w_combT", [KC, GP], F32, kind="ExternalInput")
    d_bcell = nc.dram_tensor("b_cell", [GP, 1], F32, kind="ExternalInput")
    d_h0 = nc.dram_tensor("h0", [HID, BL], F32, kind="ExternalInput")
    d_c0 = nc.dram_tensor("c0", [HID, BL], F32, kind="ExternalInput")
    d_wpass = nc.dram_tensor("w_pass", [33, v], BF16, kind="ExternalInput")
    d_out = nc.dram_tensor("out", [T, v], F32, kind="ExternalOutput")

    groups = []                                      # (start, width) vocab groups
    cc = 0
    while cc < v:
        w = min(GROUP, v - cc)
        groups.append((cc, w))
        cc += w

    with tile.TileContext(nc) as tc, ExitStack() as ctx:
        singles = ctx.enter_context(tc.tile_pool(name="singles", bufs=1))
        # phase-shared pools allocated BEFORE the scan pools so their PSUM
        # banks / SBUF addresses are disjoint from scan tiles (no false WAR
        # deps that would serialize the overlapped projection behind the
        # whole scan).
        wpool = ctx.enter_context(tc.tile_pool(name="wp", bufs=4))
        expp = ctx.enter_context(tc.tile_pool(name="expb", bufs=3))
        obp = ctx.enter_context(tc.tile_pool(name="ob", bufs=4))
        psP = ctx.enter_context(tc.tile_pool(name="psP", bufs=2, space="PSUM"))

        ident = singles.tile([128, 128], F32)
        make_identity(nc, ident)

        w_combT = singles.tile([KC, GP], F32)
        nc.gpsimd.dma_start(w_combT, d_wcomb.ap())
        b_cell = singles.tile([GP, 1], F32)
        nc.gpsimd.dma_start(b_cell, d_bcell.ap())

        # comb: [48, 520]; cols 0..260 fwd blocks 0..64, cols 260..520 rev
        # blocks 0..64. rows 0-31 = x (embT), rows 32-47 = h state.
        comb = singles.tile([KC, 2 * REV], F32)
        # concat_aug rows: 0-15 lefts, 16-31 rights, 32 = ones (pairs with
        # the b_ho row of w_pass); bf16 for full-rate PE streaming.
        concat_aug = singles.tile([33, T], BF16)
        nc.vector.memset(concat_aug[32:33, :], 1.0)

        c_state = singles.tile([HID, 2 * BL], F32)
        nc.gpsimd.dma_start(c_state[:, 0:BL], d_c0.ap())
        nc.gpsimd.dma_start(c_state[:, BL : 2 * BL], d_c0.ap())
        nc.gpsimd.dma_start(comb[EMB:KC, 0:BL], d_h0.ap())            # fwd block 0
        nc.gpsimd.dma_start(comb[EMB:KC, 2 * REV - BL : 2 * REV], d_h0.ap())  # rev 64

        embT = singles.tile([EMB, T], F32)
        # -log(sum(exp)) per token, one fp32 column per token tile (M=0, O=1)
        neg_norm = singles.tile([128, 2], F32)
        # partials[:, 2*gi + tile] = per-group exp sums
        partials = singles.tile([128, 2 * len(groups)], F32)

        # ---------------- embedding gather (transposed via PE) ----------------
        with (
            tc.tile_pool(name="pgather", bufs=2) as pg,
            tc.tile_pool(name="ps_misc", bufs=2, space="PSUM") as psm,
        ):
            for half in range(2):
                idx_sb = pg.tile([128, 1], mybir.dt.int32)
                nc.gpsimd.dma_start(
                    idx_sb, d_idx.ap()[half * 128 : (half + 1) * 128, :]
                )
                embg = pg.tile([128, EMB], F32)
                nc.gpsimd.indirect_dma_start(
                    out=embg,
                    out_offset=None,
                    in_=d_emb.ap(),
                    in_offset=bass.IndirectOffsetOnAxis(ap=idx_sb[:, :1], axis=0),
                )
                ps_tr = psm.tile([EMB, 128], F32)
                nc.tensor.transpose(ps_tr, embg, ident)
                nc.vector.tensor_copy(embT[:, half * 128 : (half + 1) * 128], ps_tr)

        # x parts of comb: fwd block t = token t; rev block m+1 = token m
        nc.vector.tensor_copy(comb[0:EMB, 0:T], embT)
        nc.vector.tensor_copy(comb[0:EMB, REV + BL : REV + BL + T], embT)

        # ---------------- fused bidirectional LSTM scan ----------------
        with (
            tc.tile_pool(name="scan_sb", bufs=4) as ssb,
            tc.tile_pool(name="scan_ps", bufs=2, space="PSUM") as sps,
        ):
            for t in range(S):
                f_col = BL * t                       # fwd block t
                r_col = REV + BL * (S - t)           # rev block 64-t
                rhs = _two_block(comb[:, :], f_col, r_col, BL)
                g_ps = sps.tile([GP, 2 * BL], F32)
                nc.tensor.matmul(g_ps, w_combT, rhs, start=True, stop=True)

                sig = ssb.tile([96, 2 * BL], F32)       # f@0, i@32, o@64
                nc.scalar.activation(
                    sig, g_ps[0:96, :], AF.Sigmoid, bias=b_cell[0:96, :]
                )
                ct = ssb.tile([48, 2 * BL], F32)        # tanh(z_C) @ 32
                nc.scalar.activation(
                    ct[32:48, :], g_ps[96:112, :], AF.Tanh, bias=b_cell[96:112, :]
                )
                f_g = sig[0:HID, :]
                i_g = sig[32 : 32 + HID, :]
                o_g = sig[64 : 64 + HID, :]

                d1 = ssb.tile([48, 2 * BL], F32)
                nc.vector.tensor_tensor(d1[32:48, :], f_g, c_state[:, :], ALU.mult)
                d2 = ssb.tile([48, 2 * BL], F32)
                nc.vector.tensor_tensor(d2[32:48, :], i_g, ct[32:48, :], ALU.mult)
                nc.vector.tensor_tensor(
                    c_state[:, :], d1[32:48, :], d2[32:48, :], ALU.add
                )

                th = ssb.tile([80, 2 * BL], F32)        # tanh(C_new) @ 64
                nc.scalar.activation(th[64:80, :], c_state[:, :], AF.Tanh)

                # h = tanh(C_new) * o -> fwd block t+1, rev block 63-t
                fo_col = BL * (t + 1)
                ro_col = REV + BL * (S - 1 - t)
                h_out = _two_block(comb[EMB:KC, :], fo_col, ro_col, BL)
                nc.vector.scalar_tensor_tensor(
                    h_out, th[64:80, :], 0.0, o_g, ALU.add, ALU.mult
                )

        # ---------------- projection ----------------
        # concat_aug column order is PERMUTED so each token tile is one
        # contiguous 128-column block (matmul stationary APs must be simple):
        #   cols   0..128 = tile M (s 16..47)
        #   cols 128..192 = s 0..15, cols 192..256 = s 48..63 (tile O)
        def copy_feats(src_lo, src_hi, dst_lo):
            """lefts/rights for comb cols [src_lo, src_hi) -> concat_aug
            (bf16 cast). rights lands at partition 16 (not 32-aligned) ->
            DMA, which also casts on the SWDGE path."""
            w = src_hi - src_lo
            nc.vector.tensor_copy(
                concat_aug[0:HID, dst_lo : dst_lo + w], comb[EMB:KC, src_lo:src_hi]
            )
            nc.gpsimd.dma_start(
                concat_aug[HID : 2 * HID, dst_lo : dst_lo + w],
                comb[EMB:KC, REV + BL + src_lo : REV + BL + src_hi],
            )

        def lhs_ap(tile_idx):
            return concat_aug[:, tile_idx * 128 : (tile_idx + 1) * 128]

        def out_ap(tile_idx, cstart, cw):
            bse = d_out.ap()
            if tile_idx == 0:
                return bass.AP(
                    bse.tensor, bse.offset + HALF * v + cstart, [[v, 128], [1, cw]]
                )
            return bass.AP(
                bse.tensor,
                bse.offset + cstart,
                [[3 * HALF * v, 2], [v, HALF], [1, cw]],
            )

        def phase_a(tile_idx):
            lhs = lhs_ap(tile_idx)
            for gi, (cstart, cw) in enumerate(groups):
                wc = wpool.tile([33, GROUP], BF16, tag="wc", name=f"wcA{tile_idx}")
                nc.scalar.dma_start(wc[:, :cw], d_wpass.ap()[:, cstart : cstart + cw])
                ps = psP.tile([128, GROUP], F32, tag="ps", name=f"psA{tile_idx}")
                for j0 in range(0, cw, 512):
                    jw = min(512, cw - j0)
                    nc.tensor.matmul(
                        ps[:, j0 : j0 + jw],
                        lhs,
                        wc[:, j0 : j0 + jw],
                        start=True,
                        stop=True,
                    )
                eb = expp.tile([128, GROUP], F32, tag="eb", name=f"eb{tile_idx}")
                pcol = 2 * gi + tile_idx
                nc.scalar.activation(
                    eb[:, :cw],
                    ps[:, :cw],
                    AF.Exp,
                    accum_out=partials[:, pcol : pcol + 1],
                )
            # neg_norm[:, tile] = -log(sum of partials)
            s_sum = expp.tile([128, 1], F32, tag="ssum", name=f"ss{tile_idx}")
            psrc = bass.AP(
                partials.tensor,
                partials.offset + tile_idx,
                [partials.ap[0], [2, len(groups)]],
            )
            nc.vector.tensor_reduce(s_sum, psrc, axis=mybir.AxisListType.X, op=ALU.add)
            ln_s = expp.tile([128, 1], F32, tag="ssum", name=f"ln{tile_idx}")
            nc.scalar.activation(ln_s, s_sum, AF.Ln)
            nc.vector.tensor_scalar_mul(
                neg_norm[:, tile_idx : tile_idx + 1], ln_s, -1.0
            )

        def phase_b(tile_idx):
            lhs = lhs_ap(tile_idx)
            nnc = neg_norm[:, tile_idx : tile_idx + 1]
            for gi, (cstart, cw) in enumerate(groups):
                wc = wpool.tile([33, GROUP], BF16, tag="wc", name=f"wcB{tile_idx}")
                nc.scalar.dma_start(wc[:, :cw], d_wpass.ap()[:, cstart : cstart + cw])
                ps = psP.tile([128, GROUP], F32, tag="ps", name=f"psB{tile_idx}")
                for j0 in range(0, cw, 512):
                    jw = min(512, cw - j0)
                    nc.tensor.matmul(
                        ps[:, j0 : j0 + jw],
                        lhs,
                        wc[:, j0 : j0 + jw],
                        start=True,
                        stop=True,
                    )
                # PSUM -> SBUF with the fp32 log-softmax shift as per-partition
                # bias; alternate ACT/DVE to balance engine load
                ob = obp.tile([128, GROUP], F32, tag="ob", name=f"ob{tile_idx}")
                if gi % 2 == 0:
                    nc.scalar.add(ob[:, :cw], ps[:, :cw], nnc)
                else:
                    nc.vector.tensor_scalar_add(ob[:, :cw], ps[:, :cw], nnc)
                nc.sync.dma_start(out_ap(tile_idx, cstart, cw), ob[:, :cw])

        # tile M (s 16..47) is ready ~3/4 through the scan -> overlaps it
        copy_feats(HALF, 3 * HALF, 0)
        phase_a(0)
        copy_feats(0, HALF, 2 * HALF)
        copy_feats(3 * HALF, T, 3 * HALF)
        phase_b(0)
        phase_a(1)
        phase_b(1)

    nc.compile()
    return nc


def host_prep(inputs, v=V, ncores=NCORES):
    """Build the per-core input maps from the full problem inputs."""
    import ml_dtypes

    emb = np.ascontiguousarray(np.asarray(inputs["embedding"], dtype=np.float32))
    ib = np.asarray(inputs["input_batch"]).astype(np.int32)          # [S, B]
    W = [np.asarray(inputs[k], dtype=np.float32) for k in ("W_f", "W_i", "W_o", "W_C")]
    b = [np.asarray(inputs[k], dtype=np.float32) for k in ("b_f", "b_i", "b_o", "b_C")]
    W_ho = np.asarray(inputs["W_ho"], dtype=np.float32)
    b_ho = np.asarray(inputs["b_ho"], dtype=np.float32)
    h0 = np.asarray(inputs["initial_hidden"], dtype=np.float32)      # [1, HID]
    c0i = np.asarray(inputs["initial_C"], dtype=np.float32)

    # padded gate layout: f@0, i@32, o@64, c@96 (tanh gate)
    Wc = np.zeros((GP, KC), dtype=np.float32)
    bc = np.zeros((GP, 1), dtype=np.float32)
    for gi, (Wg, bg) in enumerate(zip(W, b)):
        Wc[32 * gi : 32 * gi + HID] = Wg
        bc[32 * gi : 32 * gi + HID, 0] = bg
    w_combT = np.ascontiguousarray(Wc.T)             # [48, 128]

    w_pass = np.empty((33, v), dtype=np.float32)
    w_pass[0:EMB] = W_ho.T                           # [32, V]
    w_pass[EMB] = b_ho                               # pairs with the ones row
    w_pass = np.ascontiguousarray(w_pass.astype(ml_dtypes.bfloat16))

    h0T = np.ascontiguousarray(np.broadcast_to(h0.T, (HID, BL))).astype(np.float32)
    c0T = np.ascontiguousarray(np.broadcast_to(c0i.T, (HID, BL))).astype(np.float32)

    bl = B // ncores
    in_maps = []
    for c in range(ncores):
        idx = np.ascontiguousarray(
            ib[:, c * bl : (c + 1) * bl].reshape(T, 1)
        )  # token t = s*BL + b
        in_maps.append(
            {
                "emb_table": emb,
                "idx": idx,
                "w_combT": w_combT,
                "b_cell": np.ascontiguousarray(bc),
                "h0": h0T,
                "c0": c0T,
                "w_pass": w_pass,
            }
        )
    return in_maps


_NC_CACHE = {}


def kernel(**inputs):
    from concourse.bass_utils import run_bass_kernel_spmd

    if "full" not in _NC_CACHE:
        _NC_CACHE["full"] = build_nc()
    nc = _NC_CACHE["full"]
    in_maps = host_prep(inputs)
    res = run_bass_kernel_spmd(nc, in_maps, core_ids=list(range(NCORES)))
    outs = [r["out"].reshape(S, BL, V) for r in res.results]
    return np.concatenate(outs, axis=1)


# revision 15
# speedup vs baseline: 2.1083x; 2.1083x over previous
"""BiLSTM language model kernel for Trainium2 (8 NeuronCores).

Sharding: data-parallel over batch (B=32 -> 4 per core). Each core runs the
full bidirectional LSTM scan for its batch slice and computes the full-vocab
output projection + log-softmax for its tokens locally (no collectives).

Structure per core:
  - embedding rows gathered by indirect DMA, transposed via PE into a [32, T]
    "embT"; a single [48, 520] "comb" tile holds x (rows 0-31) and the h state
    (rows 32-47) for both scan directions; its h rows after the scan ARE the
    lefts/rights features.
  - fused bidirectional scan: one matmul + 3 ACT + 4 DVE ops per step handle
    both directions at once (each op acts on [*, 8] = fwd|rev batch columns).
  - output projection in two passes over vocab chunks, bf16 matmuls (PE fp32
    streams at 1/4 rate), K=33 with a ones-row paired against b_ho so the
    bias rides in the matmul. Pass A computes sum(exp) via the ACT accum_out
    port; pass B re-runs the matmuls and applies the fp32 -log(sum(exp)) as
    per-partition bias during the PSUM->SBUF copy, then DMAs out.
  - token tiles are middle-out: tile M = s 16..47 is finished by both scan
    directions ~3/4 through the scan, so its projection overlaps the scan
    tail; tile O = s {0..15, 48..63} follows. Phases pipeline as
    A(M) -> B(M) || A(O) -> B(O).

DVE/ACT constraints honored throughout: access patterns must start at
32-aligned partitions, and 2-tensor DVE ops need both SBUF inputs at the
same base partition (gates land as {f@0,i@32,o@64} with tanh intermediates
parked at matching bases).
"""

import numpy as np
from contextlib import ExitStack

import concourse.bass as bass
import concourse.mybir as mybir
import concourse.tile as tile
from concourse import bacc
from concourse.masks import make_identity

F32 = mybir.dt.float32
BF16 = mybir.dt.bfloat16
AF = mybir.ActivationFunctionType
ALU = mybir.AluOpType

S = 64          # sequence length
B = 32          # full batch
V = 50257       # vocab
HID = 16
EMB = 32
NCORES = 8
BL = B // NCORES          # batch per core = 4
T = S * BL                # tokens per core = 256
KC = EMB + HID            # 48
GP = 128                  # padded gate rows (f@0, i@32, o@64, c@96)
REV = (S + 1) * BL        # column offset of reverse region in comb = 260
GROUP = 2048              # vocab elements per psum group (4 banks)
HALF = 16 * BL            # 64 columns = 16 s-steps


def _two_block(ap2d, col_a, col_b, width):
    """AP selecting two `width`-column blocks [P, 2, width] of a 2D sbuf AP."""
    base = ap2d
    return bass.AP(
        base.tensor,
        base.offset + col_a,
        [base.ap[0], [col_b - col_a, 2], [1, width]],
    )


def build_nc(v=V):
    """Build the per-core Bass module. `v` is overridable for simulator tests."""
    nc = bacc.Bacc("TRN2", target_bir_lowering=False, debug=False)

    # ---------------- DRAM I/O ----------------
    d_emb = nc.dram_tensor("emb_table", [v, EMB], F32, kind="ExternalInput")
    d_idx = nc.dram_tensor("idx", [T, 1], mybir.dt.int32, kind="ExternalInput")
    d_wcomb = nc.dram_tensor("w_combT", [KC, GP], F32, kind="ExternalInput")
    d_bcell = nc.dram_tensor("b_cell", [GP, 1], F32, kind="ExternalInput")
    d_h0 = nc.dram_tensor("h0", [HID, BL], F32, kind="ExternalInput")
    d_c0 = nc.dram_tensor("c0", [HID, BL], F32, kind="ExternalInput")
    d_wpass = nc.dram_tensor("w_pass", [33, v], BF16, kind="ExternalInput")
    d_out = nc.dram_tensor("out", [T, v], F32, kind="ExternalOutput")

    groups = []                                      # (start, width) vocab groups
    cc = 0
    while cc < v:
        w = min(GROUP, v - cc)
        groups.append((cc, w))
        cc += w

    with tile.TileContext(nc) as tc, ExitStack() as ctx:
        singles = ctx.enter_context(tc.tile_pool(name="singles", bufs=1))
        expp = ctx.enter_context(tc.tile_pool(name="expb", bufs=3))
        obp = ctx.enter_context(tc.tile_pool(name="ob", bufs=4))

        ident = singles.tile([128, 128], F32)
        make_identity(nc, ident)

        w_combT = singles.tile([KC, GP], F32)
        nc.gpsimd.dma_start(w_combT, d_wcomb.ap())
        b_cell = singles.tile([GP, 1], F32)
        nc.gpsimd.dma_start(b_cell, d_bcell.ap())

        # comb: [48, 520]; cols 0..260 fwd blocks 0..64, cols 260..520 rev
        # blocks 0..64. rows 0-31 = x (embT), rows 32-47 = h state.
        comb = singles.tile([KC, 2 * REV], F32)
        # concat_aug rows: 0-15 lefts, 16-31 rights, 32 = ones (pairs with
        # the b_ho row of w_pass); bf16 for full-rate PE streaming.
        concat_aug = singles.tile([33, T], BF16)
        nc.vector.memset(concat_aug[32:33, :], 1.0)

        c_state = singles.tile([HID, 2 * BL], F32)
        nc.gpsimd.dma_start(c_state[:, 0:BL], d_c0.ap())
        nc.gpsimd.dma_start(c_state[:, BL : 2 * BL], d_c0.ap())
        nc.gpsimd.dma_start(comb[EMB:KC, 0:BL], d_h0.ap())            # fwd block 0
        nc.gpsimd.dma_start(comb[EMB:KC, 2 * REV - BL : 2 * REV], d_h0.ap())  # rev 64

        embT = singles.tile([EMB, T], F32)
        # the whole projection weight stays resident in SBUF (bf16,
        # ~100.5 KB/partition on 33 partitions); one DMA, issued up front so
        # it overlaps the scan
        w_sb = singles.tile([33, v], BF16)
        nc.sync.dma_start(w_sb, d_wpass.ap())
        # -log(sum(exp)) per token, one fp32 column per token tile (M=0, O=1)
        neg_norm = singles.tile([128, 2], F32)
        # partials[:, 2*gi + tile] = per-group exp sums
        partials = singles.tile([128, 2 * len(groups)], F32)

        # ---------------- embedding gather (transposed via PE) ----------------
        with (
            tc.tile_pool(name="pgather", bufs=2) as pg,
            tc.tile_pool(name="ps_misc", bufs=2, space="PSUM") as psm,
        ):
            for half in range(2):
                idx_sb = pg.tile([128, 1], mybir.dt.int32)
                nc.gpsimd.dma_start(
                    idx_sb, d_idx.ap()[half * 128 : (half + 1) * 128, :]
                )
                embg = pg.tile([128, EMB], F32)
                nc.gpsimd.indirect_dma_start(
                    out=embg,
                    out_offset=None,
                    in_=d_emb.ap(),
                    in_offset=bass.IndirectOffsetOnAxis(ap=idx_sb[:, :1], axis=0),
                )
                ps_tr = psm.tile([EMB, 128], F32)
                nc.tensor.transpose(ps_tr, embg, ident)
                nc.vector.tensor_copy(embT[:, half * 128 : (half + 1) * 128], ps_tr)

        # x parts of comb: fwd block t = token t; rev block m+1 = token m
        nc.vector.tensor_copy(comb[0:EMB, 0:T], embT)
        nc.vector.tensor_copy(comb[0:EMB, REV + BL : REV + BL + T], embT)

        # ---------------- fused bidirectional LSTM scan ----------------
        with (
            tc.tile_pool(name="scan_sb", bufs=4) as ssb,
            tc.tile_pool(name="scan_ps", bufs=2, space="PSUM") as sps,
        ):
            for t in range(S):
                f_col = BL * t                       # fwd block t
                r_col = REV + BL * (S - t)           # rev block 64-t
                rhs = _two_block(comb[:, :], f_col, r_col, BL)
                g_ps = sps.tile([GP, 2 * BL], F32)
                nc.tensor.matmul(g_ps, w_combT, rhs, start=True, stop=True)

                sig = ssb.tile([96, 2 * BL], F32)       # f@0, i@32, o@64
                nc.scalar.activation(
                    sig, g_ps[0:96, :], AF.Sigmoid, bias=b_cell[0:96, :]
                )
                ct = ssb.tile([48, 2 * BL], F32)        # tanh(z_C) @ 32
                nc.scalar.activation(
                    ct[32:48, :], g_ps[96:112, :], AF.Tanh, bias=b_cell[96:112, :]
                )
                f_g = sig[0:HID, :]
                i_g = sig[32 : 32 + HID, :]
                o_g = sig[64 : 64 + HID, :]

                d1 = ssb.tile([48, 2 * BL], F32)
                nc.vector.tensor_tensor(d1[32:48, :], f_g, c_state[:, :], ALU.mult)
                d2 = ssb.tile([48, 2 * BL], F32)
                nc.vector.tensor_tensor(d2[32:48, :], i_g, ct[32:48, :], ALU.mult)
                nc.vector.tensor_tensor(
                    c_state[:, :], d1[32:48, :], d2[32:48, :], ALU.add
                )

                th = ssb.tile([80, 2 * BL], F32)        # tanh(C_new) @ 64
                nc.scalar.activation(th[64:80, :], c_state[:, :], AF.Tanh)

                # h = tanh(C_new) * o -> fwd block t+1, rev block 63-t
                fo_col = BL * (t + 1)
                ro_col = REV + BL * (S - 1 - t)
                h_out = _two_block(comb[EMB:KC, :], fo_col, ro_col, BL)
                nc.vector.scalar_tensor_tensor(
                    h_out, th[64:80, :], 0.0, o_g, ALU.add, ALU.mult
                )

        # ---------------- projection ----------------
        # lefts/rights -> concat_aug (bf16 cast); rights lands at partition
        # 16 (not 32-aligned) -> DMA, which also casts on the SWDGE path
        nc.vector.tensor_copy(concat_aug[0:HID, :], comb[EMB:KC, 0:T])
        nc.gpsimd.dma_start(
            concat_aug[HID : 2 * HID, :], comb[EMB:KC, REV + BL : REV + BL + T]
        )

        psP = ctx.enter_context(tc.tile_pool(name="psP", bufs=2, space="PSUM"))

        def phase_a(tile_idx):
            lhs = concat_aug[:, tile_idx * 128 : (tile_idx + 1) * 128]
            for gi, (cstart, cw) in enumerate(groups):
                ps = psP.tile([128, GROUP], F32, tag="ps", name=f"psA{tile_idx}")
                for j0 in range(0, cw, 512):
                    jw = min(512, cw - j0)
                    nc.tensor.matmul(
                        ps[:, j0 : j0 + jw],
                        lhs,
                        w_sb[:, cstart + j0 : cstart + j0 + jw],
                        start=True,
                        stop=True,
                    )
                eb = expp.tile([128, GROUP], F32, tag="eb", name=f"eb{tile_idx}")
                pcol = 2 * gi + tile_idx
                nc.scalar.activation(
                    eb[:, :cw],
                    ps[:, :cw],
                    AF.Exp,
                    accum_out=partials[:, pcol : pcol + 1],
                )
            # neg_norm[:, tile] = -log(sum of partials)
            s_sum = expp.tile([128, 1], F32, tag="ssum", name=f"ss{tile_idx}")
            psrc = bass.AP(
                partials.tensor,
                partials.offset + tile_idx,
                [partials.ap[0], [2, len(groups)]],
            )
            nc.vector.tensor_reduce(s_sum, psrc, axis=mybir.AxisListType.X, op=ALU.add)
            ln_s = expp.tile([128, 1], F32, tag="ssum", name=f"ln{tile_idx}")
            nc.scalar.activation(ln_s, s_sum, AF.Ln)
            nc.vector.tensor_scalar_mul(
                neg_norm[:, tile_idx : tile_idx + 1], ln_s, -1.0
            )

        def phase_b(tile_idx):
            lhs = concat_aug[:, tile_idx * 128 : (tile_idx + 1) * 128]
            nnc = neg_norm[:, tile_idx : tile_idx + 1]
            for gi, (cstart, cw) in enumerate(groups):
                ps = psP.tile([128, GROUP], F32, tag="ps", name=f"psB{tile_idx}")
                for j0 in range(0, cw, 512):
                    jw = min(512, cw - j0)
                    nc.tensor.matmul(
                        ps[:, j0 : j0 + jw],
                        lhs,
                        w_sb[:, cstart + j0 : cstart + j0 + jw],
                        start=True,
                        stop=True,
                    )
                # PSUM -> SBUF with the fp32 log-softmax shift as per-partition
                # bias; alternate ACT/DVE to balance engine load
                ob = obp.tile([128, GROUP], F32, tag="ob", name=f"ob{tile_idx}")
                if gi % 2 == 0:
                    nc.scalar.add(ob[:, :cw], ps[:, :cw], nnc)
                else:
                    nc.vector.tensor_scalar_add(ob[:, :cw], ps[:, :cw], nnc)
                nc.sync.dma_start(
                    d_out.ap()[
                        tile_idx * 128 : (tile_idx + 1) * 128, cstart : cstart + cw
                    ],
                    ob[:, :cw],
                )

        # B(0) overlaps A(1): B(0) only waits on norm(0); A(1) is independent
        phase_a(0)
        phase_b(0)
        phase_a(1)
        phase_b(1)

    nc.compile()
    return nc


def host_prep(inputs, v=V, ncores=NCORES):
    """Build the per-core input maps from the full problem inputs."""
    import ml_dtypes

    emb = np.ascontiguousarray(np.asarray(inputs["embedding"], dtype=np.float32))
    ib = np.asarray(inputs["input_batch"]).astype(np.int32)          # [S, B]
    W = [np.asarray(inputs[k], dtype=np.float32) for k in ("W_f", "W_i", "W_o", "W_C")]
    b = [np.asarray(inputs[k], dtype=np.float32) for k in ("b_f", "b_i", "b_o", "b_C")]
    W_ho = np.asarray(inputs["W_ho"], dtype=np.float32)
    b_ho = np.asarray(inputs["b_ho"], dtype=np.float32)
    h0 = np.asarray(inputs["initial_hidden"], dtype=np.float32)      # [1, HID]
    c0i = np.asarray(inputs["initial_C"], dtype=np.float32)

    # padded gate layout: f@0, i@32, o@64, c@96 (tanh gate)
    Wc = np.zeros((GP, KC), dtype=np.float32)
    bc = np.zeros((GP, 1), dtype=np.float32)
    for gi, (Wg, bg) in enumerate(zip(W, b)):
        Wc[32 * gi : 32 * gi + HID] = Wg
        bc[32 * gi : 32 * gi + HID, 0] = bg
    w_combT = np.ascontiguousarray(Wc.T)             # [48, 128]

    w_pass = np.empty((33, v), dtype=np.float32)
    w_pass[0:EMB] = W_ho.T                           # [32, V]
    w_pass[EMB] = b_ho                               # pairs with the ones row
    w_pass = np.ascontiguousarray(w_pass.astype(ml_dtypes.bfloat16))

    h0T = np.ascontiguousarray(np.broadcast_to(h0.T, (HID, BL))).astype(np.float32)
    c0T = np.ascontiguousarray(np.broadcast_to(c0i.T, (HID, BL))).astype(np.float32)

    bl = B // ncores
    in_maps = []
    for c in range(ncores):
        idx = np.ascontiguousarray(
            ib[:, c * bl : (c + 1) * bl].reshape(T, 1)
        )  # token t = s*BL + b
        in_maps.append(
            {
                "emb_table": emb,
                "idx": idx,
                "w_combT": w_combT,
                "b_cell": np.ascontiguousarray(bc),
                "h0": h0T,
                "c0": c0T,
                "w_pass": w_pass,
            }
        )
    return in_maps


_NC_CACHE = {}


def kernel(**inputs):
    from concourse.bass_utils import run_bass_kernel_spmd

    if "full" not in _NC_CACHE:
        _NC_CACHE["full"] = build_nc()
    nc = _NC_CACHE["full"]
    in_maps = host_prep(inputs)
    res = run_bass_kernel_spmd(nc, in_maps, core_ids=list(range(NCORES)))
    outs = [r["out"].reshape(S, BL, V) for r in res.results]
    return np.concatenate(outs, axis=1)
